# revision 1
# baseline (speedup 1.0000x reference)
"""Trainium2 Bass kernel for nn_Log_Rbm.

Math: reference computes
    v = x @ W.T                               # (B, R)
    y = sum_r exp(v[:, r, None] + u[None, r, :]) + 1   summed over r
Since exp(v + u) = exp(v) * exp(u):
    y = exp(v) @ exp(u) + R
i.e. two small matmuls with elementwise exp in between — no (B, R, D_out)
intermediate is ever materialized.

Sharding (8 cores): B split 2 ways x R split 4 ways. Each core gets
  xt (D_in, 128)  = x.T columns for its B-half      (contraction on partitions)
  wt (D_in, 128)  = W.T columns for its R-quarter
  u  (128, D_out) = u rows for its R-quarter
and computes a partial y (128, D_out) = exp(wt.T@xt).T-matmul-exp(u).
Host sums the 4 R-partials per B-half and adds the scalar R (= sum of +1).
"""

import numpy as np

import concourse.bass as bass
import concourse.mybir as mybir
import concourse.tile as tile
from concourse import bacc
from concourse.bass_utils import run_bass_kernel_spmd

B, D_IN, R, D_OUT = 256, 1024, 512, 1024
P_B, P_R = 2, 4
N_CORES = P_B * P_R
B_L = B // P_B          # 128
R_L = R // P_R          # 128
K_P = 128
K_TILES = D_IN // K_P   # 8
N_SPLIT = 512           # one fp32 PSUM bank
N_TILES = D_OUT // N_SPLIT

F32 = mybir.dt.float32

_cache = {}


def _build_nc():
    nc = bacc.Bacc(
        "TRN2", target_bir_lowering=False, debug=False, enable_asserts=False
    )
    xt_d = nc.dram_tensor("xt", [D_IN, B_L], F32, kind="ExternalInput")
    wt_d = nc.dram_tensor("wt", [D_IN, R_L], F32, kind="ExternalInput")
    u_d = nc.dram_tensor("u", [R_L, D_OUT], F32, kind="ExternalInput")
    y_d = nc.dram_tensor("y", [B_L, D_OUT], F32, kind="ExternalOutput")

    xt = xt_d[:].rearrange("(k p) b -> k p b", p=K_P)
    wt = wt_d[:].rearrange("(k p) r -> k p r", p=K_P)
    u = u_d[:]
    y = y_d[:]

    EXP = mybir.ActivationFunctionType.Exp

    with tile.TileContext(nc) as tc:
        with (
            tc.tile_pool(name="sb", bufs=1) as pool,
            tc.tile_pool(name="ps", bufs=1, space="PSUM") as psum,
        ):
            # Trigger the ACT exp table-set load (~2.7us) immediately so it
            # hides under the input DMAs.
            warm = pool.tile([128, 1], F32, tag="warm")
            nc.gpsimd.memset(warm[:], 0.0)
            nc.scalar.activation(warm[:], warm[:], EXP)

            # v.T[r, b] = sum_k wt_k.T @ xt_k, accumulated in PSUM.
            vT = psum.tile([R_L, B_L], F32, tag="vT")
            for k in range(K_TILES):
                wk = pool.tile([K_P, R_L], F32, tag=f"w{k}")
                xk = pool.tile([K_P, B_L], F32, tag=f"x{k}")
                nc.sync.dma_start(wk[:], wt[k])
                nc.sync.dma_start(xk[:], xt[k])
                nc.tensor.matmul(
                    vT[:], wk[:], xk[:], start=(k == 0), stop=(k == K_TILES - 1)
                )

            # eu[r, d] = exp(u), chunked so ACT overlaps the DMA.
            u_t = pool.tile([R_L, D_OUT], F32, tag="u")
            eu = pool.tile([R_L, D_OUT], F32, tag="eu")
            for n in range(N_TILES):
                sl = bass.ts(n, N_SPLIT)
                nc.sync.dma_start(u_t[:, sl], u[:, sl])
                nc.scalar.activation(eu[:, sl], u_t[:, sl], EXP)

            evT = pool.tile([R_L, B_L], F32, tag="evT")
            nc.scalar.activation(evT[:], vT[:], EXP)

            # y[b, d] = evT.T @ eu, N chunked per PSUM bank; DMA out per chunk.
            for n in range(N_TILES):
                sl = bass.ts(n, N_SPLIT)
                yp = psum.tile([B_L, N_SPLIT], F32, tag=f"yp{n}")
                nc.tensor.matmul(yp[:], evT[:], eu[:, sl], start=True, stop=True)
                ysb = pool.tile([B_L, N_SPLIT], F32, tag=f"ysb{n}")
                nc.vector.tensor_copy(ysb[:], yp[:])
                nc.sync.dma_start(y[:, sl], ysb[:])

    nc.compile()
    return nc


def _get_nc():
    if "nc" not in _cache:
        _cache["nc"] = _build_nc()
    return _cache["nc"]


def run(x, W, u, trace=False, **spmd_kwargs):
    x = np.ascontiguousarray(x, dtype=np.float32)
    W = np.ascontiguousarray(W, dtype=np.float32)
    u = np.ascontiguousarray(u, dtype=np.float32)
    assert x.shape == (B, D_IN) and W.shape == (R, D_IN) and u.shape == (R, D_OUT)

    xt_full = np.ascontiguousarray(x.T)  # (D_IN, B)
    wt_full = np.ascontiguousarray(W.T)  # (D_IN, R)

    in_maps = []
    for core in range(N_CORES):
        ib, ir = divmod(core, P_R)
        in_maps.append(
            {
                "xt": np.ascontiguousarray(xt_full[:, ib * B_L : (ib + 1) * B_L]),
                "wt": np.ascontiguousarray(wt_full[:, ir * R_L : (ir + 1) * R_L]),
                "u": np.ascontiguousarray(u[ir * R_L : (ir + 1) * R_L, :]),
            }
        )

    nc = _get_nc()
    res = run_bass_kernel_spmd(
        nc, in_maps, core_ids=list(range(N_CORES)), trace=trace, **spmd_kwargs
    )

    out = np.empty((B, D_OUT), dtype=np.float32)
    for ib in range(P_B):
        acc = res.results[ib * P_R]["y"].copy()
        for ir in range(1, P_R):
            acc += res.results[ib * P_R + ir]["y"]
        out[ib * B_L : (ib + 1) * B_L] = acc + np.float32(R)
    return out, res


def kernel(x, W, u):
    out, _ = run(x, W, u, trace=False)
    return out


# revision 2
# speedup vs baseline: 1.5454x; 1.5454x over previous
"""Trainium2 Bass kernel for nn_Log_Rbm.

Math: reference computes
    v = x @ W.T                               # (B, R)
    y = sum_r exp(v[:, r, None] + u[None, r, :]) + 1   summed over r
Since exp(v + u) = exp(v) * exp(u):
    y = exp(v) @ exp(u) + R
i.e. two small matmuls with elementwise exp in between — no (B, R, D_out)
intermediate is ever materialized.

Sharding (8 cores): B split 2 ways x R split 4 ways. Each core gets
  xt (128, 8*128) bf16 = x.T for its B-half, pre-packed to SBUF layout
                         [p, k, b] with the contraction chunk k folded in
  wt (128, 8*128) bf16 = W.T for its R-quarter, same packing
  u  (128, 1024)  bf16 = u rows for its R-quarter
and computes a partial y (128, D_out) f32. Host sums the 4 R-partials per
B-half and adds the scalar R (= the +1 summed over R).

bf16 inputs halve DMA traffic and run the PE at 1 cycle/col instead of
fp32's 4; accumulation stays fp32 in PSUM and exp runs fp32 internally.
"""

import numpy as np
import ml_dtypes

import concourse.bass as bass
import concourse.mybir as mybir
import concourse.tile as tile
from concourse import bacc
from concourse.bass_utils import run_bass_kernel_spmd

B, D_IN, R, D_OUT = 256, 1024, 512, 1024
P_B, P_R = 2, 4
N_CORES = P_B * P_R
B_L = B // P_B          # 128
R_L = R // P_R          # 128
K_P = 128
K_TILES = D_IN // K_P   # 8
N_SPLIT = 512           # one fp32 PSUM bank
N_TILES = D_OUT // N_SPLIT

F32 = mybir.dt.float32
BF16 = mybir.dt.bfloat16
NP_BF16 = ml_dtypes.bfloat16

_cache = {}


def _build_nc():
    nc = bacc.Bacc(
        "TRN2", target_bir_lowering=False, debug=False, enable_asserts=False
    )
    xt_d = nc.dram_tensor("xt", [K_P, D_IN], BF16, kind="ExternalInput")
    wt_d = nc.dram_tensor("wt", [K_P, D_IN], BF16, kind="ExternalInput")
    u_d = nc.dram_tensor("u", [R_L, D_OUT], BF16, kind="ExternalInput")
    y_d = nc.dram_tensor("y", [B_L, D_OUT], F32, kind="ExternalOutput")

    EXP = mybir.ActivationFunctionType.Exp
    ts = bass.ts

    with tile.TileContext(nc) as tc:
        with (
            tc.tile_pool(name="sb", bufs=1) as pool,
            tc.tile_pool(name="ps", bufs=1, space="PSUM") as psum,
        ):
            # Trigger the ACT exp table-set load (~1.3us) at t=0 so it
            # hides under the input DMAs.
            warm = pool.tile([128, 1], F32, tag="warm")
            nc.gpsimd.memset(warm[:], 0.0)
            nc.scalar.activation(warm[:], warm[:], EXP)

            # One fully-contiguous DMA per input tensor (host pre-packed).
            u_t = pool.tile([R_L, D_OUT], BF16, tag="u")
            nc.sync.dma_start(u_t[:], u_d[:])
            w_t = pool.tile([K_P, D_IN], BF16, tag="w")
            nc.sync.dma_start(w_t[:], wt_d[:])
            x_t = pool.tile([K_P, D_IN], BF16, tag="x")
            nc.sync.dma_start(x_t[:], xt_d[:])

            # eu[r, d] = exp(u), chunked so both halves pipeline.
            eu = pool.tile([R_L, D_OUT], BF16, tag="eu")
            for n in range(N_TILES):
                nc.scalar.activation(eu[:, ts(n, N_SPLIT)], u_t[:, ts(n, N_SPLIT)], EXP)

            # v.T[r, b] = sum_k wt_k.T @ xt_k, accumulated in PSUM.
            vT = psum.tile([R_L, B_L], F32, tag="vT")
            for k in range(K_TILES):
                nc.tensor.matmul(
                    vT[:],
                    w_t[:, ts(k, K_P)],
                    x_t[:, ts(k, K_P)],
                    start=(k == 0),
                    stop=(k == K_TILES - 1),
                )

            evT = pool.tile([R_L, B_L], BF16, tag="evT")
            nc.scalar.activation(evT[:], vT[:], EXP)

            # y[b, d] = evT.T @ eu, N chunked per PSUM bank; copy via ACT /
            # DVE in parallel, DMA out per chunk.
            for n in range(N_TILES):
                sl = ts(n, N_SPLIT)
                yp = psum.tile([B_L, N_SPLIT], F32, tag=f"yp{n}")
                nc.tensor.matmul(yp[:], evT[:], eu[:, sl], start=True, stop=True)
                ysb = pool.tile([B_L, N_SPLIT], F32, tag=f"ysb{n}")
                if n % 2 == 0:
                    nc.vector.tensor_copy(ysb[:], yp[:])
                else:
                    nc.scalar.copy(ysb[:], yp[:])
                nc.sync.dma_start(y_d[:, sl], ysb[:])

    nc.compile()
    return nc


def _get_nc():
    if "nc" not in _cache:
        _cache["nc"] = _build_nc()
    return _cache["nc"]


def _pack_kpb(a):
    """(rows=128 cols-of-T, D_in) slice of x/W -> SBUF layout [p, k*col]:
    element (p, k, c) = a[c, k*128 + p]."""
    # a: (128, D_IN) e.g. x[b_slice, :] — want out[p, k, c] = a[c, k*128+p]
    r = a.reshape(128, K_TILES, K_P)          # (c, k, p)
    return np.ascontiguousarray(r.transpose(2, 1, 0).reshape(K_P, D_IN))


def run(x, W, u, trace=False, **spmd_kwargs):
    x = np.asarray(x, dtype=np.float32)
    W = np.asarray(W, dtype=np.float32)
    u = np.asarray(u, dtype=np.float32)
    assert x.shape == (B, D_IN) and W.shape == (R, D_IN) and u.shape == (R, D_OUT)

    x16 = x.astype(NP_BF16)
    W16 = W.astype(NP_BF16)
    u16 = u.astype(NP_BF16)

    in_maps = []
    for core in range(N_CORES):
        ib, ir = divmod(core, P_R)
        in_maps.append(
            {
                "xt": _pack_kpb(x16[ib * B_L : (ib + 1) * B_L]),
                "wt": _pack_kpb(W16[ir * R_L : (ir + 1) * R_L]),
                "u": np.ascontiguousarray(u16[ir * R_L : (ir + 1) * R_L]),
            }
        )

    nc = _get_nc()
    res = run_bass_kernel_spmd(
        nc, in_maps, core_ids=list(range(N_CORES)), trace=trace, **spmd_kwargs
    )

    out = np.empty((B, D_OUT), dtype=np.float32)
    for ib in range(P_B):
        acc = res.results[ib * P_R]["y"].copy()
        for ir in range(1, P_R):
            acc += res.results[ib * P_R + ir]["y"]
        out[ib * B_L : (ib + 1) * B_L] = acc + np.float32(R)
    return out, res


def kernel(x, W, u):
    out, _ = run(x, W, u, trace=False)
    return out


# revision 3
# speedup vs baseline: 1.5702x; 1.0160x over previous
"""Trainium2 Bass kernel for nn_Log_Rbm.

Math: reference computes
    v = x @ W.T                               # (B, R)
    y = sum_r exp(v[:, r, None] + u[None, r, :]) + 1   summed over r
Since exp(v + u) = exp(v) * exp(u):
    y = exp(v) @ exp(u) + R
i.e. two small matmuls with elementwise exp in between — no (B, R, D_out)
intermediate is ever materialized.

Sharding (8 cores): B split 2 ways x R split 4 ways. Each core gets
  xt (128, 8*128) bf16 = x.T for its B-half, pre-packed to SBUF layout
                         [p, k, b] with the contraction chunk k folded in
  wt (128, 8*128) bf16 = W.T for its R-quarter, same packing
  u  (128, 1024)  bf16 = u rows for its R-quarter
and computes a partial y (128, D_out) f32. Host sums the 4 R-partials per
B-half and adds the scalar R (= the +1 summed over R).

bf16 inputs halve DMA traffic and run the PE at 1 cycle/col instead of
fp32's 4; accumulation stays fp32 in PSUM and exp runs fp32 internally.
"""

import numpy as np
import ml_dtypes

import concourse.bass as bass
import concourse.mybir as mybir
import concourse.tile as tile
from concourse import bacc
from concourse.bass_utils import run_bass_kernel_spmd

B, D_IN, R, D_OUT = 256, 1024, 512, 1024
P_B, P_R = 2, 4
N_CORES = P_B * P_R
B_L = B // P_B          # 128
R_L = R // P_R          # 128
K_P = 128
K_TILES = D_IN // K_P   # 8
N_SPLIT = 512           # one fp32 PSUM bank
N_TILES = D_OUT // N_SPLIT

F32 = mybir.dt.float32
BF16 = mybir.dt.bfloat16
NP_BF16 = ml_dtypes.bfloat16

_cache = {}


def _build_nc():
    nc = bacc.Bacc(
        "TRN2", target_bir_lowering=False, debug=False, enable_asserts=False
    )
    xt_d = nc.dram_tensor("xt", [K_P, D_IN], BF16, kind="ExternalInput")
    wt_d = nc.dram_tensor("wt", [K_P, D_IN], BF16, kind="ExternalInput")
    u_d = nc.dram_tensor("u", [R_L, D_OUT], BF16, kind="ExternalInput")
    y_d = nc.dram_tensor("y", [B_L, D_OUT], F32, kind="ExternalOutput")

    EXP = mybir.ActivationFunctionType.Exp
    ts = bass.ts

    with tile.TileContext(nc) as tc:
        with (
            tc.tile_pool(name="sb", bufs=1) as pool,
            tc.tile_pool(name="ps", bufs=1, space="PSUM") as psum,
        ):
            # One fully-contiguous DMA per input tensor (host pre-packed),
            # each on its own DGE ring so issue + completion overlap:
            #   wt -> SP HWDGE, xt -> Activation HWDGE, u -> GpSimd SWDGE.
            w_t = pool.tile([K_P, D_IN], BF16, tag="w")
            nc.sync.dma_start(w_t[:], wt_d[:])
            x_t = pool.tile([K_P, D_IN], BF16, tag="x")
            nc.scalar.dma_start(x_t[:], xt_d[:])
            u_t = pool.tile([R_L, D_OUT], BF16, tag="u")
            nc.gpsimd.dma_start(u_t[:], u_d[:])

            # Trigger the ACT exp table-set load (~1.3us) right after the
            # xt DMA issue so it hides under the input DMA streams.
            warm = pool.tile([128, 1], F32, tag="warm")
            nc.gpsimd.memset(warm[:], 0.0)
            nc.scalar.activation(warm[:], warm[:], EXP)

            # eu[r, d] = exp(u), chunked so both halves pipeline.
            eu = pool.tile([R_L, D_OUT], BF16, tag="eu")
            for n in range(N_TILES):
                nc.scalar.activation(eu[:, ts(n, N_SPLIT)], u_t[:, ts(n, N_SPLIT)], EXP)

            # v.T[r, b] = sum_k wt_k.T @ xt_k, accumulated in PSUM.
            vT = psum.tile([R_L, B_L], F32, tag="vT")
            for k in range(K_TILES):
                nc.tensor.matmul(
                    vT[:],
                    w_t[:, ts(k, K_P)],
                    x_t[:, ts(k, K_P)],
                    start=(k == 0),
                    stop=(k == K_TILES - 1),
                )

            evT = pool.tile([R_L, B_L], BF16, tag="evT")
            nc.scalar.activation(evT[:], vT[:], EXP)

            # y[b, d] = evT.T @ eu, N chunked per PSUM bank; copies split
            # across DVE and ACT, out-DMAs split across both HWDGE rings.
            for n in range(N_TILES):
                sl = ts(n, N_SPLIT)
                yp = psum.tile([B_L, N_SPLIT], F32, tag=f"yp{n}")
                nc.tensor.matmul(yp[:], evT[:], eu[:, sl], start=True, stop=True)
                ysb = pool.tile([B_L, N_SPLIT], F32, tag=f"ysb{n}")
                if n % 2 == 0:
                    nc.vector.tensor_copy(ysb[:], yp[:])
                    nc.sync.dma_start(y_d[:, sl], ysb[:])
                else:
                    nc.scalar.copy(ysb[:], yp[:])
                    nc.scalar.dma_start(y_d[:, sl], ysb[:])

    nc.compile()
    return nc


def _get_nc():
    if "nc" not in _cache:
        _cache["nc"] = _build_nc()
    return _cache["nc"]


def _pack_kpb(a):
    """(rows=128 cols-of-T, D_in) slice of x/W -> SBUF layout [p, k*col]:
    element (p, k, c) = a[c, k*128 + p]."""
    # a: (128, D_IN) e.g. x[b_slice, :] — want out[p, k, c] = a[c, k*128+p]
    r = a.reshape(128, K_TILES, K_P)          # (c, k, p)
    return np.ascontiguousarray(r.transpose(2, 1, 0).reshape(K_P, D_IN))


def run(x, W, u, trace=False, **spmd_kwargs):
    x = np.asarray(x, dtype=np.float32)
    W = np.asarray(W, dtype=np.float32)
    u = np.asarray(u, dtype=np.float32)
    assert x.shape == (B, D_IN) and W.shape == (R, D_IN) and u.shape == (R, D_OUT)

    x16 = x.astype(NP_BF16)
    W16 = W.astype(NP_BF16)
    u16 = u.astype(NP_BF16)

    in_maps = []
    for core in range(N_CORES):
        ib, ir = divmod(core, P_R)
        in_maps.append(
            {
                "xt": _pack_kpb(x16[ib * B_L : (ib + 1) * B_L]),
                "wt": _pack_kpb(W16[ir * R_L : (ir + 1) * R_L]),
                "u": np.ascontiguousarray(u16[ir * R_L : (ir + 1) * R_L]),
            }
        )

    nc = _get_nc()
    res = run_bass_kernel_spmd(
        nc, in_maps, core_ids=list(range(N_CORES)), trace=trace, **spmd_kwargs
    )

    out = np.empty((B, D_OUT), dtype=np.float32)
    for ib in range(P_B):
        acc = res.results[ib * P_R]["y"].copy()
        for ir in range(1, P_R):
            acc += res.results[ib * P_R + ir]["y"]
        out[ib * B_L : (ib + 1) * B_L] = acc + np.float32(R)
    return out, res


def kernel(x, W, u):
    out, _ = run(x, W, u, trace=False)
    return out
